# revision 33
# baseline (speedup 1.0000x reference)
"""Sequence-parallel dense attention kernel for 8 Trainium2 NeuronCores.

Math (reference):
    h = x @ W1.T + b1                  [N, H]
    q/k/v = h @ W{q,k,v}.T + b{q,k,v}  [N, H]
    A = softmax(q @ k.T / sqrt(H))     [N, N]
    out = (h + A @ v) @ W2.T + b2      [N]

Algebraic restructuring:
  * out[n] = h[n]@w2 + (A_un[n,:]@z)/(A_un[n,:]@1) + (b_v@w2 + b2), where
    A_un = exp(scores) and z = v_nobias @ w2.  Softmax rows sum to one, so
    the v-bias contributes a constant and W2 (H->1) can be applied to V
    *before* attention — the whole [N,N]@[N,H] P@V matmul collapses into a
    [z | ones] reduction the PE does while streaming exp-scores once.
  * k = x @ (k_w @ lin1_w).T + (k_w@b1 + k_b): the k-projection is folded
    into one host-side weight so k.T is computed straight from x.T,
    concurrently with h.T — the all-gather input is ready ~25us in.
  * z = h @ (v_w.T @ w2): v is never materialized.

Sharding: rows of x across 8 cores (S = N/8 per core).  Scores are computed
transposed (ST[nk, nq] = k @ q.T) so the contraction of exp(ST) over nk is a
plain PE matmul (nk on partitions).  k.T (bf16) and z are all-gathered
(0.53MB/core, one packed AllGather).
"""

import numpy as np

N, D, H = 8192, 1024, 256
NC = 8
S = N // NC          # rows per core
NKC = N // 128       # 64 global nk chunks
SCALE = 0.0625       # 1/sqrt(256)

_cache = {}


def _build_program():
    import concourse.tile as tile
    from concourse import bacc, mybir
    from concourse.masks import make_identity

    f32 = mybir.dt.float32
    f32r = mybir.dt.float32r
    bf16 = mybir.dt.bfloat16
    Ident = mybir.ActivationFunctionType.Identity
    Exp = mybir.ActivationFunctionType.Exp

    nc = bacc.Bacc("TRN2", target_bir_lowering=False, debug=False, num_devices=NC)

    xT = nc.dram_tensor("xT", [D, S], f32r, kind="ExternalInput").ap()
    w1T = nc.dram_tensor("w1T", [D, H], f32r, kind="ExternalInput").ap()
    wk1T = nc.dram_tensor("wk1T", [D, H], f32r, kind="ExternalInput").ap()
    wqT = nc.dram_tensor("wqT", [H, H], f32r, kind="ExternalInput").ap()
    # packed small constants (per-partition columns):
    #   0-1 b1 | 2-3 bq | 4-5 bkk=k_w@b1+k_b | 6-7 w2 | 8 c0 | 9-10 wv2=v_w.T@w2
    #   11 zc0=wv2@b1
    cpk = nc.dram_tensor("cpk", [128, 16], f32, kind="ExternalInput").ap()
    # zw = lin1_w.T @ wv2 packed per d-chunk: col 2*dc = zw chunk, col 2*dc+1 = 0
    zwp = nc.dram_tensor("zwp", [128, 16], f32r, kind="ExternalInput").ap()
    out_d = nc.dram_tensor("out", [1, S], f32, kind="ExternalOutput").ap()

    HS = S // 2
    cc_inA = nc.dram_tensor("cc_inA", [H + 1, HS], bf16).ap()
    cc_inB = nc.dram_tensor("cc_inB", [H + 1, HS], bf16).ap()
    cc_outA = nc.dram_tensor("cc_outA", [(H + 1) * NC, HS], bf16, addr_space="Shared").ap()
    cc_outB = nc.dram_tensor("cc_outB", [(H + 1) * NC, HS], bf16, addr_space="Shared").ap()

    with tile.TileContext(nc) as tc:
        with (
            tc.tile_pool(name="consts", bufs=1) as consts,
            tc.tile_pool(name="xpool", bufs=8) as xpool,
            tc.tile_pool(name="work", bufs=1) as work,
            tc.tile_pool(name="small", bufs=2) as small,
            tc.tile_pool(name="expp", bufs=11) as expp,
            tc.tile_pool(name="zrp", bufs=11) as zrp,
            tc.tile_pool(name="stp", bufs=3, space="PSUM") as stp,
            tc.tile_pool(name="redp", bufs=1, space="PSUM") as redp,
        ):
            # ---- interleaved chunk loads: PE can start after the first chunk ----
            w1sb = consts.tile([128, 8, H], f32r)
            wk1sb = consts.tile([128, 8, H], f32r)
            w1c = w1T.rearrange("(c p) h -> p c h", p=128)
            wk1c = wk1T.rearrange("(c p) h -> p c h", p=128)
            xts = []
            for dc in range(8):
                nc.sync.dma_start(out=wk1sb[:, dc, :], in_=wk1c[:, dc, :])
                xt = xpool.tile([128, S], f32r, tag="xt")
                nc.sync.dma_start(out=xt, in_=xT[dc * 128:(dc + 1) * 128, :])
                xts.append(xt)
            cpack = consts.tile([128, 16], f32)
            nc.sync.dma_start(out=cpack, in_=cpk)
            zwsb = consts.tile([128, 16], f32r)
            nc.sync.dma_start(out=zwsb, in_=zwp)
            # warm the ACT exp table set before any real activation needs it
            dumm = consts.tile([1, 1], f32)
            nc.vector.memset(dumm, 0.0)
            dumo = consts.tile([1, 1], f32)
            nc.scalar.activation(out=dumo, in_=dumm, func=Exp)

            # ---- ktloc = Wk1 @ x.T + bkk and z = zw @ x.T + zc0 (no h needed) ----
            ktloc = work.tile([128, 2, S], bf16)
            zrowsb = work.tile([1, S], bf16)
            for nt in range(2):
                for hc in range(2):
                    ps = stp.tile([128, 512], f32, tag="st", name="ps")
                    for dc in range(8):
                        nc.tensor.matmul(
                            ps,
                            lhsT=wk1sb[:, dc, hc * 128:(hc + 1) * 128],
                            rhs=xts[dc][:, nt * 512:(nt + 1) * 512],
                            start=(dc == 0),
                            stop=(dc == 7),
                        )
                    nc.scalar.activation(
                        out=ktloc[:, hc, nt * 512:(nt + 1) * 512], in_=ps,
                        func=Ident, bias=cpack[:, 4 + hc:4 + hc + 1],
                    )
                psz = stp.tile([2, 512], f32, tag="st", name="psz")
                for dc in range(8):
                    nc.tensor.matmul(
                        psz,
                        lhsT=zwsb[:, 2 * dc:2 * dc + 2],
                        rhs=xts[dc][:, nt * 512:(nt + 1) * 512],
                        start=(dc == 0),
                        stop=(dc == 7),
                    )
                nc.scalar.activation(
                    out=zrowsb[:, nt * 512:(nt + 1) * 512], in_=psz[0:1, :],
                    func=Ident, bias=cpack[0:1, 11:12],
                )

            # ---- ship k.T + z, split into nk-halves so the loop can start
            # after the first half-gather while the second drains ----
            for cci, lo in ((cc_inA, 0), (cc_inB, HS)):
                for hc in range(2):
                    nc.sync.dma_start(
                        out=cci[hc * 128:(hc + 1) * 128, :],
                        in_=ktloc[:, hc, lo:lo + HS],
                    )
                nc.sync.dma_start(out=cci[H:H + 1, :], in_=zrowsb[:, lo:lo + HS])
            nc.gpsimd.collective_compute(
                "AllGather",
                mybir.AluOpType.bypass,
                replica_groups=[list(range(NC))],
                ins=[cc_inA[:]],
                outs=[cc_outA[:]],
            )
            nc.gpsimd.collective_compute(
                "AllGather",
                mybir.AluOpType.bypass,
                replica_groups=[list(range(NC))],
                ins=[cc_inB[:]],
                outs=[cc_outB[:]],
            )

            # ---- hT, q.T and residual overlap the collective ----
            for dc in range(8):
                nc.sync.dma_start(out=w1sb[:, dc, :], in_=w1c[:, dc, :])
            hTsb = work.tile([128, 2, S], f32r)
            for hc in range(2):
                for nt in range(2):
                    ps = stp.tile([128, 512], f32, tag="st", name="ps")
                    for dc in range(8):
                        nc.tensor.matmul(
                            ps,
                            lhsT=w1sb[:, dc, hc * 128:(hc + 1) * 128],
                            rhs=xts[dc][:, nt * 512:(nt + 1) * 512],
                            start=(dc == 0),
                            stop=(dc == 7),
                        )
                    nc.scalar.activation(
                        out=hTsb[:, hc, nt * 512:(nt + 1) * 512], in_=ps,
                        func=Ident, bias=cpack[:, hc:hc + 1],
                    )
            wqsb = consts.tile([128, 2, H], f32r)
            nc.sync.dma_start(out=wqsb, in_=wqT.rearrange("(c p) h -> p c h", p=128))
            ident = consts.tile([128, 128], f32)
            make_identity(nc, ident)
            zcat = consts.tile([128, 128], bf16)
            identb = consts.tile([8, 8], bf16)
            nc.vector.tensor_copy(out=identb, in_=ident[0:8, 0:8])
            onesb = consts.tile([128, 64], f32)
            nc.vector.memset(onesb, 1.0)
            nc.vector.tensor_copy(out=zcat[:, 64:128], in_=onesb)
            onesrep = zcat[:, 64:128]

            qTsb = work.tile([128, 2, S], bf16)
            for hc in range(2):
                for nt in range(2):
                    ps = stp.tile([128, 512], f32, tag="st", name="ps")
                    for hic in range(2):
                        nc.tensor.matmul(
                            ps,
                            lhsT=wqsb[:, hic, hc * 128:(hc + 1) * 128],
                            rhs=hTsb[:, hic, nt * 512:(nt + 1) * 512],
                            start=(hic == 0),
                            stop=(hic == 1),
                        )
                    nc.scalar.activation(
                        out=qTsb[:, hc, nt * 512:(nt + 1) * 512], in_=ps,
                        func=Ident, bias=cpack[:, 2 + hc:2 + hc + 1],
                    )

            residsb = consts.tile([1, S], f32)
            for nt in range(2):
                psr = stp.tile([1, 512], f32, tag="st", name="psr")
                for hic in range(2):
                    nc.tensor.matmul(
                        psr,
                        lhsT=cpack[:, 6 + hic:7 + hic],
                        rhs=hTsb[:, hic, nt * 512:(nt + 1) * 512].bitcast(f32),
                        start=(hic == 0),
                        stop=(hic == 1),
                    )
                nc.vector.tensor_copy(out=residsb[:, nt * 512:(nt + 1) * 512], in_=psr)

            # ---- unpack gathered z into zcat columns (via PE transposes) ----
            cc3A = cc_outA.rearrange("(r q) j -> r q j", q=H + 1)
            cc3B = cc_outB.rearrange("(r q) j -> r q j", q=H + 1)
            zrows = work.tile([8, S], bf16)
            nc.sync.dma_start(out=zrows[:, 0:HS], in_=cc3A[:, H, :])
            nc.sync.dma_start(out=zrows[:, HS:S], in_=cc3B[:, H, :])
            zcatf = consts.tile([128, 64], f32)
            zcv = zcat[:, 0:64].rearrange("p (j f) -> p f j", f=8)
            zcvf = zcatf.rearrange("p (j f) -> p f j", f=8)
            for f in range(8):
                pzt = stp.tile([128, 8], bf16, tag="st", name="pzt")
                nc.tensor.transpose(
                    out=pzt, in_=zrows[:, f * 128:(f + 1) * 128],
                    identity=identb[:],
                )
                nc.vector.tensor_copy(out=zcv[:, f, :], in_=pzt)
                nc.vector.tensor_copy(out=zcvf[:, f, :], in_=pzt)

            # ---- load full k.T ----
            kt0 = work.tile([128, N], bf16)
            kt1 = work.tile([128, N], bf16)
            kts = (kt0, kt1)
            for r in range(NC):
                for hc in range(2):
                    nc.sync.dma_start(
                        out=kts[hc][:, r * S:r * S + HS],
                        in_=cc3A[r, hc * 128:(hc + 1) * 128, :],
                    )
            for r in range(NC):
                for hc in range(2):
                    nc.sync.dma_start(
                        out=kts[hc][:, r * S + HS:(r + 1) * S],
                        in_=cc3B[r, hc * 128:(hc + 1) * 128, :],
                    )

            # ---- main loop: ST = k @ q.T, exp, reduce against [z | 1] ----
            psred = [
                redp.tile([128, 512], f32, tag=f"red{nt}", name=f"psred{nt}")
                for nt in range(2)
            ]
            zcr = zcat.rearrange("p (two c) -> p c two", two=2)
            exps = {}

            def emit_reduce(g, is_first, is_last):
                e = exps.pop(g)
                # replicate [z_g | 1] to a full 128-wide stationary so the
                # reduce runs in the same 128x128 tile mode as the score
                # matmuls (no PE array mode-switch drains).  Output rows
                # alternate num/den copies; rows 0/1 are read.
                zr = zrp.tile([128, 128], bf16, tag="zrep", name="zr")
                zr2 = zr.rearrange("p (c two) -> p two c", two=2)
                nc.vector.tensor_scalar_mul(zr2[:, 0, :], onesrep, zcatf[:, g:g + 1])
                nc.vector.tensor_copy(out=zr2[:, 1, :], in_=onesrep)
                for nt in range(2):
                    nc.tensor.matmul(
                        psred[nt],
                        lhsT=zr,
                        rhs=e[:, nt * 512:(nt + 1) * 512],
                        start=is_first,
                        stop=is_last,
                    )

            GRP = 8
            order = [r * 8 + l for l in range(8) for r in range(NC)]
            for gb in range(0, NKC, GRP):
                for gi in range(gb, gb + GRP):
                    g = order[gi]
                    st = stp.tile([128, 1024], f32, tag="st")
                    for hc in range(2):
                        for nt in range(2):
                            nc.tensor.matmul(
                                st[:, nt * 512:(nt + 1) * 512],
                                lhsT=kts[hc][:, g * 128:(g + 1) * 128],
                                rhs=qTsb[:, hc, nt * 512:(nt + 1) * 512],
                                start=(hc == 0),
                                stop=(hc == 1),
                            )
                    e = expp.tile([128, 1024], bf16, tag="expst")
                    nc.scalar.activation(out=e, in_=st, func=Exp, scale=SCALE)
                    exps[g] = e
                if gb > 0:
                    for gi in range(gb - GRP, gb):
                        emit_reduce(order[gi], gi == 0, False)
            for gi in range(NKC - GRP, NKC):
                emit_reduce(order[gi], gi == NKC - GRP and False, gi == NKC - 1)

            # ---- epilogue: out = resid + num/den + c0 ----
            outsb = consts.tile([1, S], f32)
            for nt in range(2):
                ndsb = small.tile([2, 512], f32, tag="nd")
                nc.vector.tensor_copy(out=ndsb, in_=psred[nt][0:2, :])
                densb = small.tile([1, 512], f32, tag="den")
                nc.sync.dma_start(out=densb, in_=ndsb[1:2, :])
                rden = small.tile([1, 512], f32, tag="rden")
                nc.vector.reciprocal(out=rden, in_=densb)
                m = small.tile([1, 512], f32, tag="m")
                nc.vector.tensor_mul(m, ndsb[0:1, :], rden)
                m2 = small.tile([1, 512], f32, tag="m2")
                nc.vector.tensor_add(m2, m, residsb[:, nt * 512:(nt + 1) * 512])
                nc.scalar.activation(
                    out=outsb[:, nt * 512:(nt + 1) * 512], in_=m2,
                    func=Ident, bias=cpack[0:1, 8:9],
                )
            nc.sync.dma_start(out=out_d[:], in_=outsb)

    nc.compile()
    return nc


def _get_program():
    if "nc" not in _cache:
        _cache["nc"] = _build_program()
    return _cache["nc"]


def kernel(x, lin1_w, lin1_b, q_w, q_b, k_w, k_b, v_w, v_b, lin2_w, lin2_b):
    from concourse.bass_utils import run_bass_kernel_spmd

    x = np.asarray(x, dtype=np.float32)
    lin1_w = np.asarray(lin1_w, dtype=np.float32)
    lin1_b = np.asarray(lin1_b, dtype=np.float32)
    q_w = np.asarray(q_w, dtype=np.float32)
    q_b = np.asarray(q_b, dtype=np.float32)
    k_w = np.asarray(k_w, dtype=np.float32)
    k_b = np.asarray(k_b, dtype=np.float32)
    v_w = np.asarray(v_w, dtype=np.float32)
    v_b = np.asarray(v_b, dtype=np.float32)
    lin2_w = np.asarray(lin2_w, dtype=np.float32)
    lin2_b = np.asarray(lin2_b, dtype=np.float32)

    nc = _get_program()

    wk1 = (k_w.astype(np.float64) @ lin1_w.astype(np.float64)).astype(np.float32)
    bkk = (k_w.astype(np.float64) @ lin1_b.astype(np.float64)).astype(np.float32) + k_b
    w2 = lin2_w[0]                                  # [H]
    wv2 = (v_w.T.astype(np.float64) @ w2.astype(np.float64)).astype(np.float32)
    c0 = np.float32(v_b @ w2 + lin2_b[0])

    cpk = np.zeros((128, 16), dtype=np.float32)
    cpk[:, 0:2] = lin1_b.reshape(2, 128).T
    cpk[:, 2:4] = q_b.reshape(2, 128).T
    cpk[:, 4:6] = bkk.reshape(2, 128).T
    cpk[:, 6:8] = w2.reshape(2, 128).T
    cpk[0, 8] = c0
    cpk[:, 9:11] = wv2.reshape(2, 128).T
    zw = (lin1_w.T.astype(np.float64) @ wv2.astype(np.float64)).astype(np.float32)
    zc0 = np.float32(wv2.astype(np.float64) @ lin1_b.astype(np.float64))
    cpk[0, 11] = zc0
    zwp = np.zeros((128, 16), dtype=np.float32)
    zwp[:, 0::2] = zw.reshape(8, 128).T

    w1T = np.ascontiguousarray(lin1_w.T)            # [D, H]
    wk1T = np.ascontiguousarray(wk1.T)              # [D, H]
    wqT = np.ascontiguousarray(q_w.T)               # [H, H]

    in_maps = []
    for i in range(NC):
        in_maps.append({
            "xT": np.ascontiguousarray(x[i * S:(i + 1) * S, :].T),
            "w1T": w1T, "wk1T": wk1T, "wqT": wqT,
            "cpk": cpk, "zwp": zwp,
        })

    res = run_bass_kernel_spmd(nc, in_maps, core_ids=list(range(NC)))
    out = np.concatenate([res.results[i]["out"].reshape(S) for i in range(NC)])
    return out.astype(np.float32)


# revision 36
# speedup vs baseline: 1.1131x; 1.1131x over previous
"""Sequence-parallel dense attention kernel for 8 Trainium2 NeuronCores.

Math (reference):
    h = x @ W1.T + b1                  [N, H]
    q/k/v = h @ W{q,k,v}.T + b{q,k,v}  [N, H]
    A = softmax(q @ k.T / sqrt(H))     [N, N]
    out = (h + A @ v) @ W2.T + b2      [N]

Algebraic restructuring:
  * out[n] = h[n]@w2 + (A_un[n,:]@z)/(A_un[n,:]@1) + (b_v@w2 + b2), where
    A_un = exp(scores) and z = v_nobias @ w2.  Softmax rows sum to one, so
    the v-bias contributes a constant and W2 (H->1) can be applied to V
    *before* attention — the whole [N,N]@[N,H] P@V matmul collapses into a
    [z | ones] reduction the PE does while streaming exp-scores once.
  * k = x @ (k_w @ lin1_w).T + (k_w@b1 + k_b): the k-projection is folded
    into one host-side weight so k.T is computed straight from x.T,
    concurrently with h.T — the all-gather input is ready ~25us in.
  * z = h @ (v_w.T @ w2): v is never materialized.

Sharding: rows of x across 8 cores (S = N/8 per core).  Scores are computed
transposed (ST[nk, nq] = k @ q.T) so the contraction of exp(ST) over nk is a
plain PE matmul (nk on partitions).  k.T (bf16) and z are all-gathered
(0.53MB/core, one packed AllGather).
"""

import numpy as np

N, D, H = 8192, 1024, 256
NC = 8
S = N // NC          # rows per core
NKC = N // 128       # 64 global nk chunks
SCALE = 0.0625       # 1/sqrt(256)

_cache = {}


def _build_program():
    import concourse.tile as tile
    from concourse import bacc, mybir
    from concourse.masks import make_identity

    f32 = mybir.dt.float32
    f32r = mybir.dt.float32r
    bf16 = mybir.dt.bfloat16
    Ident = mybir.ActivationFunctionType.Identity
    Exp = mybir.ActivationFunctionType.Exp
    Log = mybir.ActivationFunctionType.Ln

    nc = bacc.Bacc("TRN2", target_bir_lowering=False, debug=False, num_devices=NC)

    xT = nc.dram_tensor("xT", [D, S], f32r, kind="ExternalInput").ap()
    w1T = nc.dram_tensor("w1T", [D, H], f32r, kind="ExternalInput").ap()
    wk1T = nc.dram_tensor("wk1T", [D, H], f32r, kind="ExternalInput").ap()
    wqT = nc.dram_tensor("wqT", [H, H], f32r, kind="ExternalInput").ap()
    # packed small constants (per-partition columns):
    #   0-1 b1 | 2-3 bq | 4-5 bkk=k_w@b1+k_b | 6-7 w2 | 8 c0 | 9-10 wv2=v_w.T@w2
    #   11 zc0=wv2@b1
    cpk = nc.dram_tensor("cpk", [128, 16], f32, kind="ExternalInput").ap()
    # zw = lin1_w.T @ wv2 packed per d-chunk: col 2*dc = zw chunk, col 2*dc+1 = 0
    zwp = nc.dram_tensor("zwp", [128, 16], f32r, kind="ExternalInput").ap()
    out_d = nc.dram_tensor("out", [1, S], f32, kind="ExternalOutput").ap()

    cc_in = nc.dram_tensor("cc_in", [H + 1, S], bf16).ap()
    cc_out = nc.dram_tensor("cc_out", [(H + 1) * NC, S], bf16, addr_space="Shared").ap()

    with tile.TileContext(nc) as tc:
        with (
            tc.tile_pool(name="consts", bufs=1) as consts,
            tc.tile_pool(name="xpool", bufs=8) as xpool,
            tc.tile_pool(name="work", bufs=1) as work,
            tc.tile_pool(name="small", bufs=2) as small,
            tc.tile_pool(name="expp", bufs=11) as expp,
            tc.tile_pool(name="zrp", bufs=11) as zrp,
            tc.tile_pool(name="stp", bufs=3, space="PSUM") as stp,
            tc.tile_pool(name="redp", bufs=1, space="PSUM") as redp,
        ):
            # ---- interleaved chunk loads: PE can start after the first chunk ----
            w1sb = consts.tile([128, 8, H], f32r)
            wk1sb = consts.tile([128, 8, H], f32r)
            w1c = w1T.rearrange("(c p) h -> p c h", p=128)
            wk1c = wk1T.rearrange("(c p) h -> p c h", p=128)
            xts = []
            for dc in range(8):
                nc.sync.dma_start(out=wk1sb[:, dc, :], in_=wk1c[:, dc, :])
                xt = xpool.tile([128, S], f32r, tag="xt")
                nc.sync.dma_start(out=xt, in_=xT[dc * 128:(dc + 1) * 128, :])
                xts.append(xt)
            cpack = consts.tile([128, 16], f32)
            nc.sync.dma_start(out=cpack, in_=cpk)
            zwsb = consts.tile([128, 16], f32r)
            nc.sync.dma_start(out=zwsb, in_=zwp)
            # warm the ACT exp table set before any real activation needs it
            dumm = consts.tile([1, 1], f32)
            nc.vector.memset(dumm, 0.0)
            dumo = consts.tile([1, 1], f32)
            nc.scalar.activation(out=dumo, in_=dumm, func=Exp)

            # ---- ktloc = Wk1 @ x.T + bkk and z = zw @ x.T + zc0 (no h needed) ----
            ktloc = work.tile([128, 2, S], bf16)
            zrowsb = work.tile([1, S], bf16)
            for nt in range(2):
                for hc in range(2):
                    ps = stp.tile([128, 512], f32, tag="st", name="ps")
                    for dc in range(8):
                        nc.tensor.matmul(
                            ps,
                            lhsT=wk1sb[:, dc, hc * 128:(hc + 1) * 128],
                            rhs=xts[dc][:, nt * 512:(nt + 1) * 512],
                            start=(dc == 0),
                            stop=(dc == 7),
                        )
                    nc.scalar.activation(
                        out=ktloc[:, hc, nt * 512:(nt + 1) * 512], in_=ps,
                        func=Ident, bias=cpack[:, 4 + hc:4 + hc + 1],
                    )
                psz = stp.tile([2, 512], f32, tag="st", name="psz")
                for dc in range(8):
                    nc.tensor.matmul(
                        psz,
                        lhsT=zwsb[:, 2 * dc:2 * dc + 2],
                        rhs=xts[dc][:, nt * 512:(nt + 1) * 512],
                        start=(dc == 0),
                        stop=(dc == 7),
                    )
                nc.scalar.activation(
                    out=zrowsb[:, nt * 512:(nt + 1) * 512], in_=psz[0:1, :],
                    func=Ident, bias=cpack[0:1, 11:12],
                )

            # ---- ship k.T + z to collective input ----
            for hc in range(2):
                nc.sync.dma_start(
                    out=cc_in[hc * 128:(hc + 1) * 128, :], in_=ktloc[:, hc, :]
                )
            nc.sync.dma_start(out=cc_in[H:H + 1, :], in_=zrowsb)

            # ---- all-gather k.T + z (1MB + 4KB per rank) ----
            nc.gpsimd.collective_compute(
                "AllGather",
                mybir.AluOpType.bypass,
                replica_groups=[list(range(NC))],
                ins=[cc_in[:]],
                outs=[cc_out[:]],
            )

            # ---- hT, q.T and residual overlap the collective ----
            for dc in range(8):
                nc.sync.dma_start(out=w1sb[:, dc, :], in_=w1c[:, dc, :])
            hTsb = work.tile([128, 2, S], f32r)
            for hc in range(2):
                for nt in range(2):
                    ps = stp.tile([128, 512], f32, tag="st", name="ps")
                    for dc in range(8):
                        nc.tensor.matmul(
                            ps,
                            lhsT=w1sb[:, dc, hc * 128:(hc + 1) * 128],
                            rhs=xts[dc][:, nt * 512:(nt + 1) * 512],
                            start=(dc == 0),
                            stop=(dc == 7),
                        )
                    nc.scalar.activation(
                        out=hTsb[:, hc, nt * 512:(nt + 1) * 512], in_=ps,
                        func=Ident, bias=cpack[:, hc:hc + 1],
                    )
            wqsb = consts.tile([128, 2, H], f32r)
            nc.sync.dma_start(out=wqsb, in_=wqT.rearrange("(c p) h -> p c h", p=128))
            ident = consts.tile([128, 128], f32)
            make_identity(nc, ident)
            zcat = consts.tile([128, 128], bf16)
            identb = consts.tile([8, 8], bf16)
            nc.vector.tensor_copy(out=identb, in_=ident[0:8, 0:8])
            onesb = consts.tile([128, 64], f32)
            nc.vector.memset(onesb, 1.0)
            nc.vector.tensor_copy(out=zcat[:, 64:128], in_=onesb)
            onesrep = zcat[:, 64:128]

            qTsb = work.tile([128, 2, S], bf16)
            for hc in range(2):
                for nt in range(2):
                    ps = stp.tile([128, 512], f32, tag="st", name="ps")
                    for hic in range(2):
                        nc.tensor.matmul(
                            ps,
                            lhsT=wqsb[:, hic, hc * 128:(hc + 1) * 128],
                            rhs=hTsb[:, hic, nt * 512:(nt + 1) * 512],
                            start=(hic == 0),
                            stop=(hic == 1),
                        )
                    nc.scalar.activation(
                        out=qTsb[:, hc, nt * 512:(nt + 1) * 512], in_=ps,
                        func=Ident, bias=cpack[:, 2 + hc:2 + hc + 1],
                    )

            residsb = consts.tile([1, S], f32)
            for nt in range(2):
                psr = stp.tile([1, 512], f32, tag="st", name="psr")
                for hic in range(2):
                    nc.tensor.matmul(
                        psr,
                        lhsT=cpack[:, 6 + hic:7 + hic],
                        rhs=hTsb[:, hic, nt * 512:(nt + 1) * 512].bitcast(f32),
                        start=(hic == 0),
                        stop=(hic == 1),
                    )
                nc.vector.tensor_copy(out=residsb[:, nt * 512:(nt + 1) * 512], in_=psr)

            # ---- unpack gathered z into zcat columns (via PE transposes) ----
            cc3 = cc_out.rearrange("(r q) j -> r q j", q=H + 1)
            zrows = work.tile([8, S], bf16)
            nc.sync.dma_start(out=zrows, in_=cc3[:, H, :])
            zcatf = consts.tile([128, 64], f32)
            zcv = zcat[:, 0:64].rearrange("p (j f) -> p f j", f=8)
            zcvf = zcatf.rearrange("p (j f) -> p f j", f=8)
            for f in range(8):
                pzt = stp.tile([128, 8], bf16, tag="st", name="pzt")
                nc.tensor.transpose(
                    out=pzt, in_=zrows[:, f * 128:(f + 1) * 128],
                    identity=identb[:],
                )
                nc.vector.tensor_copy(out=zcv[:, f, :], in_=pzt)
                nc.vector.tensor_copy(out=zcvf[:, f, :], in_=pzt)

            # ---- load full k.T ----
            kt0 = work.tile([128, N], bf16)
            kt1 = work.tile([128, N], bf16)
            kts = (kt0, kt1)
            for r in range(NC):
                for hc in range(2):
                    nc.sync.dma_start(
                        out=kts[hc][:, r * S:(r + 1) * S],
                        in_=cc3[r, hc * 128:(hc + 1) * 128, :],
                    )

            # ---- main loop: ST = k @ q.T, exp, reduce against [z | 1] ----
            psred = [
                redp.tile([128, 512], f32, tag=f"red{nt}", name=f"psred{nt}")
                for nt in range(2)
            ]
            zcr = zcat.rearrange("p (two c) -> p c two", two=2)
            exps = {}

            def emit_reduce(g):
                e = exps.pop(g)
                # replicate [z_g | 1] to a full 128-wide stationary so the
                # reduce runs in the same 128x128 tile mode as the score
                # matmuls (no PE array mode-switch drains).  Output rows
                # alternate num/den copies; rows 0/1 are read.
                zr = zrp.tile([128, 128], bf16, tag="zrep", name="zr")
                zr2 = zr.rearrange("p (c two) -> p two c", two=2)
                nc.vector.tensor_scalar_mul(zr2[:, 0, :], onesrep, zcatf[:, g:g + 1])
                nc.vector.tensor_copy(out=zr2[:, 1, :], in_=onesrep)
                for nt in range(2):
                    nc.tensor.matmul(
                        psred[nt],
                        lhsT=zr,
                        rhs=e[:, nt * 512:(nt + 1) * 512],
                        start=(g == 0),
                        stop=(g == NKC - 1),
                    )

            GRP = 8
            for gb in range(0, NKC, GRP):
                for g in range(gb, gb + GRP):
                    st = stp.tile([128, 1024], f32, tag="st")
                    for hc in range(2):
                        for nt in range(2):
                            nc.tensor.matmul(
                                st[:, nt * 512:(nt + 1) * 512],
                                lhsT=kts[hc][:, g * 128:(g + 1) * 128],
                                rhs=qTsb[:, hc, nt * 512:(nt + 1) * 512],
                                start=(hc == 0),
                                stop=(hc == 1),
                            )
                    e = expp.tile([128, 1024], bf16, tag="expst")
                    nc.scalar.activation(out=e, in_=st, func=Exp, scale=SCALE)
                    exps[g] = e
                if gb > 0:
                    for g in range(gb - GRP, gb):
                        emit_reduce(g)
            for g in range(NKC - GRP, NKC):
                emit_reduce(g)

            # ---- epilogue: out = resid + num/den + c0 ----
            outsb = consts.tile([1, S], f32)
            for nt in range(2):
                ndsb = small.tile([2, 512], f32, tag="nd")
                nc.vector.tensor_copy(out=ndsb, in_=psred[nt][0:2, :])
                densb = small.tile([1, 512], f32, tag="den")
                nc.sync.dma_start(out=densb, in_=ndsb[1:2, :])
                lnd = small.tile([1, 512], f32, tag="lnd")
                nc.scalar.activation(out=lnd, in_=densb, func=Log)
                rden = small.tile([1, 512], f32, tag="rden")
                nc.scalar.activation(out=rden, in_=lnd, func=Exp, scale=-1.0)
                m = small.tile([1, 512], f32, tag="m")
                nc.vector.tensor_mul(m, ndsb[0:1, :], rden)
                m2 = small.tile([1, 512], f32, tag="m2")
                nc.vector.tensor_add(m2, m, residsb[:, nt * 512:(nt + 1) * 512])
                nc.scalar.activation(
                    out=outsb[:, nt * 512:(nt + 1) * 512], in_=m2,
                    func=Ident, bias=cpack[0:1, 8:9],
                )
            nc.sync.dma_start(out=out_d[:], in_=outsb)

    nc.compile()
    return nc


def _get_program():
    if "nc" not in _cache:
        _cache["nc"] = _build_program()
    return _cache["nc"]


def kernel(x, lin1_w, lin1_b, q_w, q_b, k_w, k_b, v_w, v_b, lin2_w, lin2_b):
    from concourse.bass_utils import run_bass_kernel_spmd

    x = np.asarray(x, dtype=np.float32)
    lin1_w = np.asarray(lin1_w, dtype=np.float32)
    lin1_b = np.asarray(lin1_b, dtype=np.float32)
    q_w = np.asarray(q_w, dtype=np.float32)
    q_b = np.asarray(q_b, dtype=np.float32)
    k_w = np.asarray(k_w, dtype=np.float32)
    k_b = np.asarray(k_b, dtype=np.float32)
    v_w = np.asarray(v_w, dtype=np.float32)
    v_b = np.asarray(v_b, dtype=np.float32)
    lin2_w = np.asarray(lin2_w, dtype=np.float32)
    lin2_b = np.asarray(lin2_b, dtype=np.float32)

    nc = _get_program()

    wk1 = (k_w.astype(np.float64) @ lin1_w.astype(np.float64)).astype(np.float32)
    bkk = (k_w.astype(np.float64) @ lin1_b.astype(np.float64)).astype(np.float32) + k_b
    w2 = lin2_w[0]                                  # [H]
    wv2 = (v_w.T.astype(np.float64) @ w2.astype(np.float64)).astype(np.float32)
    c0 = np.float32(v_b @ w2 + lin2_b[0])

    cpk = np.zeros((128, 16), dtype=np.float32)
    cpk[:, 0:2] = lin1_b.reshape(2, 128).T
    cpk[:, 2:4] = q_b.reshape(2, 128).T
    cpk[:, 4:6] = bkk.reshape(2, 128).T
    cpk[:, 6:8] = w2.reshape(2, 128).T
    cpk[0, 8] = c0
    cpk[:, 9:11] = wv2.reshape(2, 128).T
    zw = (lin1_w.T.astype(np.float64) @ wv2.astype(np.float64)).astype(np.float32)
    zc0 = np.float32(wv2.astype(np.float64) @ lin1_b.astype(np.float64))
    cpk[0, 11] = zc0
    zwp = np.zeros((128, 16), dtype=np.float32)
    zwp[:, 0::2] = zw.reshape(8, 128).T

    w1T = np.ascontiguousarray(lin1_w.T)            # [D, H]
    wk1T = np.ascontiguousarray(wk1.T)              # [D, H]
    wqT = np.ascontiguousarray(q_w.T)               # [H, H]

    in_maps = []
    for i in range(NC):
        in_maps.append({
            "xT": np.ascontiguousarray(x[i * S:(i + 1) * S, :].T),
            "w1T": w1T, "wk1T": wk1T, "wqT": wqT,
            "cpk": cpk, "zwp": zwp,
        })

    res = run_bass_kernel_spmd(nc, in_maps, core_ids=list(range(NC)))
    out = np.concatenate([res.results[i]["out"].reshape(S) for i in range(NC)])
    return out.astype(np.float32)
